# revision 5
# baseline (speedup 1.0000x reference)
"""MultiHeadAttentionBlock (B=2, S=2048, D=1024, H=16, causal) on 8 trn2 cores.

Sharding: tensor-parallel over heads (2 heads / core) for QKV projections and
attention; an on-device AllToAll redistributes the per-head context so each
core computes the full output projection for its 512-row block. The host only
slices / transposes / casts inputs and concatenates the 8 row-blocks.

Per-core dataflow (fp32 PSUM accumulation everywhere), interleaved per batch
so batch-1 projections fill PE gaps while batch-0 attention waits on ScalarE
exp:
  Q/K projections run as fp8e4m3 DoubleRow matmuls: x in fp8, weights split
  w = (w_hi + w_lo/64) with both parts stored prescaled by 64 in fp8 (the
  residual would underflow e4m3's subnormal floor unscaled), accumulating
  64*(x@w) in PSUM over 4 contraction supertiles of 256; the DVE evict does
  (psum * 1/64 + bias) and writes Q^T/K^T as fp8 [128, 2048] (dims on
  partitions).
  V is projected straight into its PV-natural layout [rows, dims] per
  128-row tile in bf16 (8 k-tile matmuls + a ones x b_v rank-1 matmul for the
  free-dim bias), evicted with a single strided DVE copy into V3 whose
  per-head ones-column accumulates the softmax denominator in PSUM for free.
  S^T[j,i] = K_j^T Q_i per (batch, head) as a DoubleRow matmul with a
  stride-0 broadcast duplicating the dk=64 contraction (computes 2*K^T Q at
  0.5 cycles/row; the doubling is folded into the exp scale 1/16). exp on
  ScalarE; scores are small so no max-subtraction is needed; triangular mask
  on the diagonal block via VectorE; PV in bf16 with partial-width matmuls on
  the diagonal (sub-diagonal columns never touched); normalize with
  reciprocal + gpsimd partition_broadcast (source must sit in partition 0 --
  the HW ucode broadcasts partition 0 literally) + VectorE multiply.
  AllToAll (gpsimd collective) with per-destination-chunk receive DMAs, then
  out = ctx^T.T @ w_o^T + b_o streamed per 128-row tile.
"""

import os
import numpy as np
import ml_dtypes

B, S, D = 2, 2048, 1024
H, DK = 16, 64
ROWS = B * S  # 4096
NCORES = 8
CDIM = 128  # context dims per core (2 heads x 64)
RPC = ROWS // NCORES  # 512 rows per core

BF16 = ml_dtypes.bfloat16
FP8 = ml_dtypes.float8_e4m3

_CACHE = {}
LAST_RESULTS = None  # stashed BassKernelResults for external inspection


def _build_program(with_collective=True):
    import concourse.mybir as mybir
    import concourse.tile as tile
    from concourse import bacc

    f32 = mybir.dt.float32
    bf = mybir.dt.bfloat16
    f8 = mybir.dt.float8e4
    Exp = mybir.ActivationFunctionType.Exp
    DR = mybir.MatmulPerfMode.DoubleRow
    mult = mybir.AluOpType.mult
    add = mybir.AluOpType.add

    nc = bacc.Bacc(
        "TRN2", target_bir_lowering=False, debug=False, num_devices=NCORES
    )

    # --- per-core DRAM I/O ---
    xqT_d = nc.dram_tensor("xqT", [D, ROWS], f8, kind="ExternalInput").ap()
    xkT_d = nc.dram_tensor("xkT", [D, ROWS], f8, kind="ExternalInput").ap()
    xvT_d = nc.dram_tensor("xvT", [D, ROWS], bf, kind="ExternalInput").ap()
    wqh_d = nc.dram_tensor("wqh", [D, CDIM], f8, kind="ExternalInput").ap()
    wql_d = nc.dram_tensor("wql", [D, CDIM], f8, kind="ExternalInput").ap()
    wkh_d = nc.dram_tensor("wkh", [D, CDIM], f8, kind="ExternalInput").ap()
    wkl_d = nc.dram_tensor("wkl", [D, CDIM], f8, kind="ExternalInput").ap()
    wvT_d = nc.dram_tensor("wvT", [D, CDIM], bf, kind="ExternalInput").ap()
    bq_d = nc.dram_tensor("bq", [CDIM, 1], f32, kind="ExternalInput").ap()
    bk_d = nc.dram_tensor("bk", [CDIM, 1], f32, kind="ExternalInput").ap()
    bvr_d = nc.dram_tensor("bvr", [1, CDIM], bf, kind="ExternalInput").ap()
    woT_d = nc.dram_tensor("woT", [D, D], bf, kind="ExternalInput").ap()
    bo_d = nc.dram_tensor("bo", [1, D], bf, kind="ExternalInput").ap()
    triu_d = nc.dram_tensor("triu", [128, 128], bf, kind="ExternalInput").ap()
    out_d = nc.dram_tensor("out", [RPC, D], f32, kind="ExternalOutput").ap()

    with tile.TileContext(nc) as tc:
        with (
            tc.tile_pool(name="sb", bufs=1) as sb,
            tc.tile_pool(name="ps", bufs=1, space="PSUM") as ps,
            tc.tile_pool(name="dram", bufs=1, space="DRAM") as dram,
        ):
            # --- constants / weights ---
            def load_w8(dram_ap, name):
                t = sb.tile([128, 8, CDIM], f8, tag="w8", bufs=4, name=name)
                nc.sync.dma_start(
                    out=t, in_=dram_ap.rearrange("(ko ki) m -> ki ko m", ki=128)
                )
                return t

            wq3h = load_w8(wqh_d, "wq3h")
            wq3l = load_w8(wql_d, "wq3l")
            wk3h = load_w8(wkh_d, "wk3h")
            wk3l = load_w8(wkl_d, "wk3l")
            bq_sb = sb.tile([CDIM, 1], f32, tag="bias", bufs=2)
            nc.sync.dma_start(out=bq_sb, in_=bq_d)
            bk_sb = sb.tile([CDIM, 1], f32, tag="bias", bufs=2)
            nc.sync.dma_start(out=bk_sb, in_=bk_d)
            wv3 = sb.tile([128, 8, CDIM], bf, tag="wv", bufs=1)
            bvr_sb = sb.tile([1, CDIM], bf, tag="bvr", bufs=1)
            bo_sb = sb.tile([1, D], bf, tag="bo", bufs=1)
            triu_sb = sb.tile([128, 128], bf, tag="triu", bufs=1)
            ones_sb = sb.tile([1, 128], bf, tag="ones", bufs=1)
            nc.vector.memset(ones_sb, 1.0)
            # preload the exp table set during the DMA ramp so the first real
            # exp doesn't pay the ~2.7us ACT_TABLE_LOAD
            warm_sb = sb.tile([128, 128], bf, tag="warm", bufs=1)
            nc.vector.memset(warm_sb, 1.0)
            nc.scalar.activation(
                out=warm_sb[0:1, 0:1], in_=ones_sb[0:1, 0:1], func=Exp, scale=1.0
            )
            wo3 = sb.tile([128, 8, D], bf, tag="wo", bufs=1)

            send_d = dram.tile([NCORES, CDIM, RPC], bf, tag="send")
            recv_d = dram.tile([NCORES, CDIM, RPC], bf, tag="recv")

            # per-batch persistent tiles: Q^T/K^T fp8 (dims on partitions),
            # V3 bf16 in PV-natural layout [V_h0 | 1 | V_h1 | 1] per key tile
            QT, KT, V3 = {}, {}, {}
            for b in range(B):
                QT[b] = sb.tile([128, S], f8, tag="qt", bufs=2, name=f"QT{b}")
                KT[b] = sb.tile([128, S], f8, tag="kt", bufs=2, name=f"KT{b}")
                V3[b] = sb.tile([128, 16, 130], bf, tag="v3", bufs=2, name=f"V3{b}")
                nc.vector.memset(V3[b][:, :, 64:65], 1.0)
                nc.vector.memset(V3[b][:, :, 129:130], 1.0)

            def load_x(x_d, dt_, tag, bufs, b, pref, ih):
                # 4 DMAs of [128, 2, 1024] (one contraction supertile each;
                # dim1 = the two 128-deep k-slices of the supertile)
                co = S * b + 1024 * ih
                ts = []
                for g in range(4):
                    t = sb.tile(
                        [128, 2, 1024], dt_, tag=tag, bufs=bufs,
                        name=f"{pref}{b}_{g}_{ih}",
                    )
                    src_ap = x_d[g * 256 : (g + 1) * 256, co : co + 1024]
                    nc.sync.dma_start(
                        out=t, in_=src_ap.rearrange("(ko ki) m -> ki ko m", ki=128)
                    )
                    ts.append(t)
                return ts

            xv_tiles = {}  # (b, ih) -> 4 supertile tiles for V

            def proj_chunk(w3h, w3l, bias_sb, outT, xts, n):
                # one 512-row chunk of the Q/K projection: 8 fp8 DoubleRow
                # matmuls accumulate 64*(x @ w) over 4 supertiles of 256;
                # evict rescales by 1/64, adds bias, writes fp8
                pt = ps.tile([128, 512], f32, tag="proj", bufs=2, name="pproj")
                cs = slice((n % 2) * 512, (n % 2) * 512 + 512)
                for g in range(4):
                    nc.tensor.matmul(
                        pt,
                        w3h[:, 2 * g : 2 * g + 2, :],
                        xts[g][:, :, cs],
                        start=(g == 0),
                        stop=False,
                        perf_mode=DR,
                    )
                for g in range(4):
                    nc.tensor.matmul(
                        pt,
                        w3l[:, 2 * g : 2 * g + 2, :],
                        xts[g][:, :, cs],
                        start=False,
                        stop=(g == 3),
                        perf_mode=DR,
                    )
                nc.vector.tensor_scalar(
                    out=outT[:, n * 512 : (n + 1) * 512],
                    in0=pt,
                    scalar1=1.0 / 64.0,
                    scalar2=bias_sb,
                    op0=mult,
                    op1=add,
                )

            def v_tile(b, rt):
                # project one 128-row V tile straight into natural layout:
                # out[row, dim] over both heads; bias enters as ones x b_v
                pv = ps.tile([128, 128], f32, tag="proj", bufs=2, name="pvt")
                xts = xv_tiles[(b, rt // 8)]
                rs = slice((rt % 8) * 128, (rt % 8) * 128 + 128)
                nc.tensor.matmul(
                    pv, ones_sb, bvr_sb, start=True, stop=False
                )
                for kk in range(8):
                    nc.tensor.matmul(
                        pv,
                        xts[kk // 2][:, kk % 2, rs],
                        wv3[:, kk, :],
                        start=False,
                        stop=(kk == 7),
                    )
                # single strided copy into V3, skipping the ones columns:
                # cols {0..63, 65..128} <- pv[:, 0:128]
                dst = V3[b][:, rt, :].rearrange("p (two c) -> p two c", two=2)
                nc.vector.tensor_copy(out=dst[:, :, 0:64], in_=pv)

            def attention(b, hl, ih, fillers):
                # fillers: jt -> list of thunks (next-phase PE work) injected
                # so TensorE stays fed while ScalarE paces the exp stream
                pb = 64 * hl
                ibase = 1024 * ih
                cps = {}
                for ic in (2 * ih, 2 * ih + 1):
                    cps[ic] = ps.tile(
                        [128, 512], f32, tag="ctx", bufs=2, name=f"cps{b}{hl}{ic}"
                    )
                for jt in range(8 * (ih + 1)):
                    for f in fillers.get(jt, ()):
                        f()
                    jpos = 128 * jt
                    if hl == 0 and jt // 8 == ih:
                        v_tile(b, jt)
                    estart = max(jpos, ibase)
                    off0 = estart - ibase
                    ex = sb.tile([128, 1024], bf, tag="ex", bufs=14, name="ex")
                    sc = ps.tile([128, 1024], f32, tag="sc", bufs=2, name="sc")
                    off = off0
                    while off < 1024:
                        cw = min(512 - off % 512, 1024 - off)
                        # DoubleRow with a stride-0 broadcast over the dk=64
                        # contraction: computes 2*K^T Q at 0.5 cycles/row
                        lhsT = (
                            KT[b][pb : pb + 64, jpos : jpos + 128]
                            .unsqueeze(1)
                            .broadcast_to([64, 2, 128])
                        )
                        rhs = (
                            QT[b][pb : pb + 64, ibase + off : ibase + off + cw]
                            .unsqueeze(1)
                            .broadcast_to([64, 2, cw])
                        )
                        nc.tensor.matmul(
                            sc[:, off : off + cw],
                            lhsT,
                            rhs,
                            start=True,
                            stop=True,
                            perf_mode=DR,
                        )
                        off += cw
                    nc.scalar.activation(
                        out=ex[:, off0:1024],
                        in_=sc[:, off0:1024],
                        func=Exp,
                        scale=0.0625,
                    )
                    if jt // 8 == ih:
                        # diagonal block lives in this i-half: mask it
                        dg = jpos - ibase
                        nc.vector.tensor_mul(
                            ex[:, dg : dg + 128], ex[:, dg : dg + 128], triu_sb
                        )
                    # PV: diagonal i-chunk is partial width (cols < jpos are
                    # masked and never touched)
                    for ic in (2 * ih, 2 * ih + 1):
                        if 512 * (ic + 1) <= jpos:
                            continue
                        lo = max(512 * ic, jpos)
                        nc.tensor.matmul(
                            cps[ic][0:65, lo - 512 * ic : 512],
                            V3[b][:, jt, 65 * hl : 65 * hl + 65],
                            ex[:, lo - ibase : 512 * (ic + 1) - ibase],
                            start=(jt == 0),
                            stop=(jt == 4 * ic + 3),
                        )
                    if jt % 4 == 3 and jt // 4 in cps:
                        # chunk ic finished accumulating: normalize (PSUM row
                        # 64 holds the softmax denominator), free its slot
                        ic = jt // 4
                        rs = sb.tile([128, 512], f32, tag="rs", bufs=4, name="rs")
                        # cross-base DVE op: read PSUM p64, write SBUF p0
                        # (partition_broadcast HW broadcasts partition 0)
                        nc.vector.reciprocal(out=rs[0:1, :], in_=cps[ic][64:65, :])
                        rb = sb.tile([64, 512], f32, tag="rb", bufs=4, name="rb")
                        nc.gpsimd.partition_broadcast(rb[0:64, :], rs[0:1, :])
                        cn = sb.tile([64, 512], bf, tag="cn", bufs=6, name="cn")
                        nc.vector.tensor_mul(
                            cn[0:64, :], cps[ic][0:64, :], rb[0:64, :]
                        )
                        nc.gpsimd.dma_start(
                            out=send_d[4 * b + ic, pb : pb + 64, :],
                            in_=cn[0:64, :],
                        )

            def spread(thunks, jts):
                return {jt: [t] for jt, t in zip(jts, thunks)}

            def mk(w3h, w3l, bias, outT, xts, n):
                return lambda: proj_chunk(w3h, w3l, bias, outT, xts, n)

            # a serial MM->copy->MM chain trickles ~1 matmul/us to keep the
            # PE p-state ramped while the prologue DMAs land
            def warm(n):
                for _ in range(n):
                    pwk = ps.tile([128, 128], f32, tag="sc", bufs=2, name="pwk")
                    nc.tensor.matmul(
                        pwk, ones_sb, warm_sb[0:1, :], start=True, stop=True
                    )
                    nc.vector.tensor_copy(out=warm_sb, in_=pwk)

            # --- software pipeline: batch-0 prologue, then each attention
            # phase carries the next phase's projections as fillers. DMA
            # issue order is by first-use time (the DMA engines saturate for
            # the first ~50us, so order is what matters). ---
            warm(6)
            nc.sync.dma_start(out=triu_sb, in_=triu_d)
            xq00 = load_x(xqT_d, f8, "xt8", 12, 0, "xq", 0)
            xk00 = load_x(xkT_d, f8, "xt8", 12, 0, "xk", 0)
            xq01 = load_x(xqT_d, f8, "xt8", 12, 0, "xq", 1)
            xk01 = load_x(xkT_d, f8, "xt8", 12, 0, "xk", 1)
            nc.sync.dma_start(
                out=wv3, in_=wvT_d.rearrange("(ko ki) m -> ki ko m", ki=128)
            )
            nc.sync.dma_start(out=bvr_sb, in_=bvr_d)
            xv_tiles[(0, 0)] = load_x(xvT_d, bf, "xtv", 12, 0, "xv", 0)
            xv_tiles[(0, 1)] = load_x(xvT_d, bf, "xtv", 12, 0, "xv", 1)
            proj_chunk(wq3h, wq3l, bq_sb, QT[0], xq00, 0)
            proj_chunk(wq3h, wq3l, bq_sb, QT[0], xq00, 1)
            proj_chunk(wk3h, wk3l, bk_sb, KT[0], xk00, 0)
            pb01 = [
                mk(wk3h, wk3l, bk_sb, KT[0], xk00, 1),  # K1: keys 512-1024
                mk(wq3h, wq3l, bq_sb, QT[0], xq01, 2),
                mk(wq3h, wq3l, bq_sb, QT[0], xq01, 3),
                mk(wk3h, wk3l, bk_sb, KT[0], xk01, 2),
                mk(wk3h, wk3l, bk_sb, KT[0], xk01, 3),
            ]
            attention(0, 0, 0, spread(pb01[:3], (1, 3, 5)))
            attention(0, 0, 1, spread(pb01[3:], (1, 4)))
            # b1 Q/K x-slices: exp-critical for the second half
            xq10 = load_x(xqT_d, f8, "xt8", 12, 1, "xq", 0)
            xk10 = load_x(xkT_d, f8, "xt8", 12, 1, "xk", 0)
            xq11 = load_x(xqT_d, f8, "xt8", 12, 1, "xq", 1)
            xk11 = load_x(xkT_d, f8, "xt8", 12, 1, "xk", 1)
            nc.sync.dma_start(out=bo_sb, in_=bo_d)
            xv_tiles[(1, 0)] = load_x(xvT_d, bf, "xtv", 12, 1, "xv", 0)
            pb1 = [
                mk(wq3h, wq3l, bq_sb, QT[1], xq10, 0),
                mk(wq3h, wq3l, bq_sb, QT[1], xq10, 1),
                mk(wk3h, wk3l, bk_sb, KT[1], xk10, 0),
                mk(wk3h, wk3l, bk_sb, KT[1], xk10, 1),
                mk(wq3h, wq3l, bq_sb, QT[1], xq11, 2),
                mk(wq3h, wq3l, bq_sb, QT[1], xq11, 3),
                mk(wk3h, wk3l, bk_sb, KT[1], xk11, 2),
                mk(wk3h, wk3l, bk_sb, KT[1], xk11, 3),
            ]
            attention(0, 1, 0, spread(pb1[:4], (1, 3, 5, 7)))
            # o-proj weights + late V slices: load in the DMA lull
            nc.sync.dma_start(
                out=wo3, in_=woT_d.rearrange("(ko ki) m -> ki ko m", ki=128)
            )
            xv_tiles[(1, 1)] = load_x(xvT_d, bf, "xtv", 12, 1, "xv", 1)
            attention(0, 1, 1, spread(pb1[4:6], (2, 6)))
            attention(1, 0, 0, spread(pb1[6:], (2, 6)))
            attention(1, 0, 1, {})
            attention(1, 1, 0, {})
            attention(1, 1, 1, {})

            # --- all-to-all: chunk r of my send goes to core r. The timed
            # stand-in copies per destination chunk fire as soon as that
            # chunk's sends land, so only the last chunk's hop is exposed;
            # receive DMAs pipeline the same way and the output projection
            # starts immediately off the still-warm PE. ---
            if with_collective:
                nc.gpsimd.collective_compute(
                    "AllToAll",
                    mybir.AluOpType.bypass,
                    replica_groups=[list(range(NCORES))],
                    ins=[send_d.opt()],
                    outs=[recv_d.opt()],
                )
            else:
                # timing-only stand-in (TimelineSim has no collectives)
                for p in range(NCORES):
                    nc.sync.dma_start(out=recv_d[p], in_=send_d[p])
            rcvp = []
            for p in range(NCORES):
                t = sb.tile([128, RPC], bf, tag="rcv", bufs=8, name=f"rcv{p}")
                nc.sync.dma_start(out=t, in_=recv_d[p])
                rcvp.append(t)

            # --- output projection for my 512-row block, streamed per
            # 128-row tile; accumulation starts as soon as rcv chunks land ---
            for it in range(4):
                po0 = ps.tile([128, 512], f32, tag="ctx", bufs=2, name="po0")
                po1 = ps.tile([128, 512], f32, tag="ctx", bufs=2, name="po1")
                nc.tensor.matmul(po0, ones_sb, bo_sb[:, 0:512], start=True, stop=False)
                nc.tensor.matmul(po1, ones_sb, bo_sb[:, 512:1024], start=True, stop=False)
                for p in range(NCORES):
                    lhsT = rcvp[p][:, it * 128 : (it + 1) * 128]
                    nc.tensor.matmul(
                        po0, lhsT, wo3[:, p, 0:512], start=False, stop=(p == 7)
                    )
                    nc.tensor.matmul(
                        po1, lhsT, wo3[:, p, 512:1024], start=False, stop=(p == 7)
                    )
                ob = sb.tile([128, D], f32, tag="ob", bufs=2, name="ob")
                nc.vector.tensor_copy(out=ob[:, 0:512], in_=po0)
                nc.vector.tensor_copy(out=ob[:, 512:1024], in_=po1)
                nc.sync.dma_start(out=out_d[it * 128 : (it + 1) * 128, :], in_=ob)

    nc.compile()
    return nc


def _prep_inputs(q, k, v, w_q, b_q, w_k, b_k, w_v, b_v, w_o, b_o):
    def bf(x):
        return np.ascontiguousarray(x).astype(BF16)

    def f8(x):
        return np.ascontiguousarray(x).astype(FP8)

    q = np.asarray(q, np.float32).reshape(ROWS, D)
    k = np.asarray(k, np.float32).reshape(ROWS, D)
    v = np.asarray(v, np.float32).reshape(ROWS, D)
    xqT = f8(q.T)
    xkT = f8(k.T)
    xvT = bf(v.T)
    w_q = np.asarray(w_q, np.float32)
    w_k = np.asarray(w_k, np.float32)
    w_v = np.asarray(w_v, np.float32)
    w_o = np.asarray(w_o, np.float32)
    woT = bf(w_o.T)
    bo = bf(np.asarray(b_o, np.float32).reshape(1, D))
    triu = np.triu(np.ones((128, 128), np.float32)).astype(BF16)

    def whl(w_c):
        # w ~= (wh + wl/64); store both prescaled by 64 (exact exponent
        # shift) so PSUM accumulates 64*(x @ w) in one group
        wh = w_c.astype(FP8)
        whs = (wh.astype(np.float32) * 64.0).astype(FP8)
        wls = ((w_c - wh.astype(np.float32)) * 64.0).astype(FP8)
        return whs, wls

    in_maps = []
    for c in range(NCORES):
        hs = slice(c * CDIM, (c + 1) * CDIM)
        wqh, wql = whl(np.ascontiguousarray(w_q[hs, :].T))
        wkh, wkl = whl(np.ascontiguousarray(w_k[hs, :].T))
        in_maps.append(
            {
                "xqT": xqT,
                "xkT": xkT,
                "xvT": xvT,
                "wqh": wqh,
                "wql": wql,
                "wkh": wkh,
                "wkl": wkl,
                "wvT": bf(w_v[hs, :].T),
                "bq": np.ascontiguousarray(
                    np.asarray(b_q, np.float32)[hs].reshape(CDIM, 1)
                ),
                "bk": np.ascontiguousarray(
                    np.asarray(b_k, np.float32)[hs].reshape(CDIM, 1)
                ),
                "bvr": bf(np.asarray(b_v, np.float32)[hs].reshape(1, CDIM)),
                "woT": woT,
                "bo": bo,
                "triu": triu,
            }
        )
    return in_maps


def kernel(q, k, v, mask, w_q, b_q, w_k, b_k, w_v, b_v, w_o, b_o):
    global LAST_RESULTS
    if "nc" not in _CACHE:
        _CACHE["nc"] = _build_program()
    nc = _CACHE["nc"]

    from concourse.bass_utils import run_bass_kernel_spmd

    in_maps = _prep_inputs(q, k, v, w_q, b_q, w_k, b_k, w_v, b_v, w_o, b_o)
    res = run_bass_kernel_spmd(nc, in_maps, core_ids=list(range(NCORES)))
    LAST_RESULTS = res
    out = np.concatenate(
        [np.asarray(res.results[c]["out"], np.float32) for c in range(NCORES)], axis=0
    )
    return out.reshape(B, S, D)
